# revision 1
# baseline (speedup 1.0000x reference)
"""DiscriminativeLoss kernel for Trainium2 (8 NeuronCores, data-parallel over batch).

Problem: nn_DiscriminativeLoss (B=8, C=4, H=512, W=1024, K=5 lanes).
One sample per core.  Each core returns 30 floats:
  cnt_k, S_kc = sum_{label=k} emb_c, varsum_k = sum_{label=k} relu(||e-m_k||-0.5)^2
The host finishes the tiny math (means, KxK centroid distances, scan) in f64.

Engine plan (per core, pixel-major [128, 4096] bf16 tiles):
  DVE : 5x TS is_equal -> masks (4x mode) with fused count accum
        20x TT mask*e products (2x mode); 5x TT mask*r2; relu via TS add/max
  PE  : all big reductions as ones-matmul chains into PSUM [1,512] rows
  ACT : 20x fused Square(e_c - m_kc) via per-partition bias; 5x Sqrt
"""

import sys

sys.path.insert(0, "/opt/trn_rl_repo")

import numpy as np
import ml_dtypes

import concourse.bass as bass
import concourse.tile as tile
from concourse import mybir
from concourse.bass_utils import run_bass_kernel_spmd


def _split_excess_waits(nc):
    """This walrus build allows 1 sync-wait per instruction (2 for
    EventSemaphore).  Tile's sem assignment can attach more; hoist the excess
    onto fresh NOPs inserted immediately before the instruction (identical
    blocking semantics on the engine's in-order stream)."""
    import bass_rust

    si_cls = bass_rust.SyncInfo
    nsplit = 0
    for bb in nc.main_func.blocks:
        insts = bb.instructions  # live, mutable list
        new_list = []
        for ins in list(insts):
            si = getattr(ins, "sync_info", None)
            cap = 2 if type(ins).__name__ == "InstEventSemaphore" else 1
            if si is not None and len(si.on_wait) > cap:
                waits = list(si.on_wait)
                for w in waits[: len(waits) - cap]:
                    nop = bass_rust.InstNoOp(
                        name=f"I-wsplit-{nc.next_id()}", text_hint="wait_split"
                    )
                    nop.engine = ins.engine
                    nop.sync_info = si_cls(on_wait=[w], on_update=[])
                    nc.register_instruction(nop)
                    new_list.append(nop)
                    nsplit += 1
                ins.sync_info = si_cls(
                    on_wait=waits[len(waits) - cap :],
                    on_update=list(si.on_update),
                )
            new_list.append(ins)
        insts[:] = new_list
    return nsplit


# ---------------------------------------------------------------------------
# Problem constants (hardcoded per the harness contract)
# ---------------------------------------------------------------------------
B, C, H, W = 8, 4, 512, 1024
K = 5
DELTA_V = 0.5
DELTA_D = 3.0
NPIX = H * W          # 524288
P = 128
FD = NPIX // P        # 4096
NCHUNK = FD // 512    # 8 matmul chunks per reduction
N_CORES = 8

BF16 = mybir.dt.bfloat16
F32 = mybir.dt.float32
A = mybir.AluOpType
AF = mybir.ActivationFunctionType

# stats0 row layout ([1, 32] partition-0 tile):  S[20] | cnt[5] | varsum[5]
COL_S = 0
COL_CNT = 20
COL_VAR = 25
N_STATS = 30

_compiled = None


def _build():
    nc = bass.Bass()
    emb_d = nc.dram_tensor("emb", [C, NPIX], BF16, kind="ExternalInput")
    lab_d = nc.dram_tensor("lab", [NPIX], BF16, kind="ExternalInput")
    out_d = nc.dram_tensor("out", [N_STATS], F32, kind="ExternalOutput")

    with tile.TileContext(nc) as tc:
        with (
            tc.tile_pool(name="persist", bufs=1) as persist,
            tc.tile_pool(name="prod", bufs=3) as prodp,
            tc.tile_pool(name="sq", bufs=6) as sqp,
            tc.tile_pool(name="dacc", bufs=3) as daccp,
            tc.tile_pool(name="small", bufs=1) as small,
            tc.tile_pool(name="ps", bufs=3, space="PSUM") as psp,
            tc.tile_pool(name="psb", bufs=1, space="PSUM") as psbp,
        ):
            # ---- loads: half-tensor DMAs alternating between the two
            # verified DGE issuers so transfers overlap ------------------
            H2 = FD // 2
            LAB = persist.tile([P, FD], BF16, tag="LAB")
            lab_ap = lab_d.rearrange("(p f) -> p f", p=P)
            nc.sync.dma_start(out=LAB[:, 0:H2], in_=lab_ap[:, 0:H2])
            nc.gpsimd.dma_start(out=LAB[:, H2:FD], in_=lab_ap[:, H2:FD])
            E = []
            for c in range(C):
                t = persist.tile([P, FD], BF16, tag=f"E{c}", name=f"E{c}")
                e_ap = emb_d[c].rearrange("(p f) -> p f", p=P)
                eng0, eng1 = (nc.sync, nc.gpsimd) if c % 2 else (nc.gpsimd, nc.sync)
                eng0.dma_start(out=t[:, 0:H2], in_=e_ap[:, 0:H2])
                eng1.dma_start(out=t[:, H2:FD], in_=e_ap[:, H2:FD])
                E.append(t)

            ones_bf = small.tile([P, 1], BF16, tag="ones_bf")
            nc.vector.memset(ones_bf[:], 1.0)
            ones_f = small.tile([P, 1], F32, tag="ones_f")
            nc.vector.memset(ones_f[:], 1.0)
            ones128 = small.tile([P, P], F32, tag="ones128")
            nc.vector.memset(ones128[:], 1.0)
            cnt128 = small.tile([P, K], F32, tag="cnt128")
            stats0 = small.tile([1, 32], F32, tag="stats0")

            def pe_reduce_to(col, src):
                """sum(src [P, FD] bf16) -> stats0[0, col] via PE + DVE."""
                pr = psp.tile([1, 512], F32, tag="pr", name="pr")
                for j in range(NCHUNK):
                    nc.tensor.matmul(
                        pr[:],
                        ones_bf[:],
                        src[:, j * 512 : (j + 1) * 512],
                        start=(j == 0),
                        stop=(j == NCHUNK - 1),
                    )
                nc.vector.tensor_reduce(
                    out=stats0[:, col : col + 1],
                    in_=pr[:],
                    axis=mybir.AxisListType.X,
                    op=A.add,
                )

            # ---- pass 1: masks (+fused counts) and masked sums ---------
            masks = []
            for k in range(1, K + 1):
                m = persist.tile([P, FD], BF16, tag=f"mask{k}", name=f"mask{k}")
                nc.vector.tensor_scalar(
                    out=m[:],
                    in0=LAB[:],
                    scalar1=float(k),
                    scalar2=0.0,
                    op0=A.is_equal,
                    op1=A.add,
                    accum_out=cnt128[:, k - 1 : k],
                )
                masks.append(m)
                for c in range(C):
                    pr_t = prodp.tile([P, FD], BF16, tag="prod", name="prod")
                    nc.vector.tensor_tensor(
                        out=pr_t[:], in0=masks[k - 1][:], in1=E[c][:], op=A.mult
                    )
                    pe_reduce_to(COL_S + 4 * (k - 1) + c, pr_t)

            # counts: [P, K] f32 -> [1, K] via one matmul
            prc = psp.tile([1, K], F32, tag="prc")
            nc.tensor.matmul(prc[:], ones_f[:], cnt128[:])
            nc.vector.tensor_copy(stats0[:, COL_CNT : COL_CNT + K], prc[:])

            # ---- means on partition 0, broadcast to all partitions -----
            rec = small.tile([1, K], F32, tag="rec")
            nc.vector.reciprocal(rec[:], stats0[:, COL_CNT : COL_CNT + K])
            negmean0 = small.tile([1, K, C], F32, tag="negmean0")
            nc.vector.tensor_tensor(
                out=negmean0[:],
                in0=stats0[:, COL_S : COL_S + K * C].rearrange(
                    "o (k c) -> o k c", k=K
                ),
                in1=bass.AP(
                    tensor=rec.tensor,
                    offset=rec.offset,
                    ap=[rec.ap[0], rec.ap[1], [0, C]],
                ),
                op=A.mult,
            )
            nc.vector.tensor_scalar(
                out=negmean0[:], in0=negmean0[:], scalar1=-1.0, scalar2=None, op0=A.mult
            )
            scal128 = small.tile([P, K * C], F32, tag="scal128")
            nc.vector.memset(scal128[:], 0.0)
            nc.vector.tensor_copy(
                scal128[0:1, :], negmean0[:].rearrange("o k c -> o (k c)")
            )
            pb = psbp.tile([P, K * C], F32, tag="pb")
            nc.tensor.matmul(pb[:], ones128[:], scal128[:])
            negmean = small.tile([P, K, C], F32, tag="negmean")
            nc.vector.tensor_copy(negmean[:].rearrange("p k c -> p (k c)"), pb[:])

            # ---- pass 2: per-lane distances and var sums ---------------
            for k in range(1, K + 1):
                sq = []
                for c in range(C):
                    t = sqp.tile([P, FD], BF16, tag="sq", name="sq")
                    nc.scalar.activation(
                        out=t[:],
                        in_=E[c][:],
                        func=AF.Square,
                        bias=negmean[:, k - 1, c : c + 1],
                        scale=1.0,
                    )
                    sq.append(t)
                # d2 = sq0+sq1+sq2+sq3 on DVE (tree)
                nc.vector.tensor_tensor(out=sq[0][:], in0=sq[0][:], in1=sq[1][:], op=A.add)
                nc.vector.tensor_tensor(out=sq[2][:], in0=sq[2][:], in1=sq[3][:], op=A.add)
                nc.vector.tensor_tensor(out=sq[0][:], in0=sq[0][:], in1=sq[2][:], op=A.add)
                dist = daccp.tile([P, FD], BF16, tag="dacc", name="dacc")
                nc.scalar.activation(out=dist[:], in_=sq[0][:], func=AF.Sqrt)
                # r2 = relu(dist - 0.5) then square, on DVE
                r = daccp.tile([P, FD], BF16, tag="dacc", name="r")
                nc.vector.tensor_scalar(
                    out=r[:],
                    in0=dist[:],
                    scalar1=-DELTA_V,
                    scalar2=0.0,
                    op0=A.add,
                    op1=A.max,
                )
                mr = prodp.tile([P, FD], BF16, tag="prod", name="mr")
                nc.vector.tensor_tensor(
                    out=mr[:], in0=r[:], in1=masks[k - 1][:], op=A.mult
                )
                mr2 = prodp.tile([P, FD], BF16, tag="prod", name="mr2")
                nc.vector.tensor_tensor(out=mr2[:], in0=mr[:], in1=mr[:], op=A.mult)
                pe_reduce_to(COL_VAR + k - 1, mr2)

            # ---- store -------------------------------------------------
            nc.sync.dma_start(
                out=out_d.rearrange("(o n) -> o n", o=1),
                in_=stats0[0:1, 0:N_STATS],
            )

    _split_excess_waits(nc)
    return nc


def _get_compiled():
    global _compiled
    if _compiled is None:
        _compiled = _build()
    return _compiled


def kernel(embedding_tensor: np.ndarray, instance_labels: np.ndarray):
    nc = _get_compiled()

    emb = np.ascontiguousarray(embedding_tensor.reshape(B, C, NPIX))
    lab = instance_labels.reshape(B, NPIX)
    lab_bf = lab.astype(np.float32).astype(ml_dtypes.bfloat16)
    emb_bf = emb.astype(ml_dtypes.bfloat16)

    in_maps = [{"emb": emb_bf[b], "lab": lab_bf[b]} for b in range(B)]
    res = run_bass_kernel_spmd(nc, in_maps, list(range(N_CORES)))

    dt = np.float64
    v = dt(0.0)
    d = dt(0.0)
    denom_v = dt(K)
    denom_d = dt(2 * K * (K - 1))
    for b in range(B):
        st = res.results[b]["out"].astype(dt)
        S = st[COL_S : COL_S + K * C].reshape(K, C)
        cnt = st[COL_CNT : COL_CNT + K]
        varsum = st[COL_VAR : COL_VAR + K]

        means = S / cnt[:, None]
        s_b = np.sum(varsum / cnt)

        cdiff = means[:, None, :] - means[None, :, :]
        cdist = np.sqrt(np.sum(cdiff * cdiff, axis=-1)) + np.eye(K, dtype=dt) * DELTA_D
        p_b = np.sum(np.maximum(DELTA_D - cdist, 0.0) ** 2)

        v = (v + s_b) / denom_v
        d = (d + p_b) / denom_d

    v = v / B
    d = d / B
    return np.float32(v), np.float32(d)

